# revision 5
# baseline (speedup 1.0000x reference)
"""GNN message-passing (Convolve) kernel for Trainium2, 8 NeuronCores.

Reference computation (B=8, N=8192, C=256, H=256, O=256, K=64):
    g   = embeddings[:, neighbor_set, :]                     # [B, K, C]
    h   = leaky_relu(g @ Qw + Qb)                            # [B, K, H]
    w   = weights[neighbor_set, node_id]                     # [K]
    s   = sum_k h * w / (sum_k w + eps)                      # [B, H]
    z   = concat(embeddings[:, node_id, :], s)               # [B, C+H]
    o   = leaky_relu(z @ Ww + Wb)                            # [B, O]
    out = o / (||o||_2 + eps)                                # [B, O]

Sharding: data-parallel over the batch axis — core b handles batch b.
Each core receives an augmented table T = [embeddings[b] | weights[:, node_id]]
([N, C+1]) so one indirect-DMA gather family fetches both the neighbor
embedding row and its edge weight.  Qw/Ww/biases are replicated.

Device dataflow (all fp32):
    gather g_ext[64, 257] (2 split indirect DMAs, k-packed idx [32, 2])
    gT chunks via PE transposes; w as column g_ext[:, 256]
    h_pre[64, 256] = gT.T @ Qw  (+ ones.T @ Qb only if Qb != 0)
    h = max(h_pre, 0.3 h_pre)   (DVE)
    s_raw cols [128,1]x2 = h[:, chunk].T @ w_col   (PE, w folded, unnormalized)
    A[1,256] = node_cols.T @ Ww[:256]              (PE, during gather window)
    B[1,256] = s_cols.T @ Ww[256:]                 (PE)
    rec = 1/(sum w + eps)  (PE transpose w->row, DVE reduce/recip, early)
    x = A + Wb + rec*B; o = max(x, .3x)            (DVE)
    out = o / (sqrt(sum o^2) + eps)                (DVE reduce, warm ACT sqrt)
"""

import functools

import numpy as np

import concourse.bacc as bacc
import concourse.bass as bass
import concourse.mybir as mybir
import concourse.tile as tile
from concourse.bass_utils import run_bass_kernel_spmd

B, N, C, H, O, K = 8, 8192, 256, 256, 256, 64
ALPHA = 0.3
EPS = 1e-6
F32 = mybir.dt.float32
I32 = mybir.dt.int32
N_CORES = 8
MAX = mybir.AluOpType.max
MULT = mybir.AluOpType.mult
ADD = mybir.AluOpType.add


def _build_program(node_id: int, has_qb: bool) -> bass.Bass:
    nc = bacc.Bacc(None, target_bir_lowering=False, debug=False)

    embw = nc.dram_tensor("embw", [N, C + 1], F32, kind="ExternalInput")
    qw = nc.dram_tensor("qw", [C, H], F32, kind="ExternalInput")
    ww = nc.dram_tensor("ww", [C + H, O], F32, kind="ExternalInput")
    wb = nc.dram_tensor("wb", [1, O], F32, kind="ExternalInput")
    nbr = nc.dram_tensor("nbr", [32, 2], I32, kind="ExternalInput")
    ident_d = nc.dram_tensor("ident", [K, K], F32, kind="ExternalInput")
    if has_qb:
        qb = nc.dram_tensor("qb", [1, H], F32, kind="ExternalInput")
        onesr = nc.dram_tensor("onesr", [1, K], F32, kind="ExternalInput")
    out_d = nc.dram_tensor("out", [1, O], F32, kind="ExternalOutput")

    with tile.TileContext(nc) as tc:
        with (
            tc.tile_pool(name="sb", bufs=1) as sb,
            tc.tile_pool(name="ps", bufs=1, space="PSUM") as ps,
        ):
            # ---- early independent loads (sync HWDGE) ----
            idx = sb.tile([32, 2], I32)
            nc.sync.dma_start(out=idx[:], in_=nbr[:])
            ident = sb.tile([K, K], F32)
            nc.sync.dma_start(out=ident[:], in_=ident_d[:])
            qw_sb = []
            for j in range(2):
                t = sb.tile([128, H], F32, tag=f"qw{j}")
                nc.sync.dma_start(out=t[:], in_=qw[128 * j : 128 * (j + 1), :])
                qw_sb.append(t)
            cc = sb.tile([1, C], F32)
            nc.sync.dma_start(out=cc[:], in_=embw[node_id : node_id + 1, 0:C])
            ww_sb = []
            for j in range(4):
                t = sb.tile([128, O], F32, tag=f"ww{j}")
                nc.sync.dma_start(out=t[:], in_=ww[128 * j : 128 * (j + 1), :])
                ww_sb.append(t)
            wb_r = sb.tile([1, O], F32)
            nc.sync.dma_start(out=wb_r[:], in_=wb[:])
            if has_qb:
                qb_r = sb.tile([1, H], F32)
                nc.sync.dma_start(out=qb_r[:], in_=qb[:])
                ones_r = sb.tile([1, K], F32)
                nc.sync.dma_start(out=ones_r[:], in_=onesr[:])

            # ---- warm the ACT sqrt table off the critical path ----
            warm = sb.tile([1, 1], F32)
            nc.scalar.activation(
                out=warm[:], in_=ident[0:1, 0:1],
                func=mybir.ActivationFunctionType.Sqrt,
            )

            # ---- gather: g_ext[k, :] = embw[idx[k], :]  (2 splits) ----
            g = sb.tile([K, C + 1], F32)
            for j in range(2):
                nc.gpsimd.indirect_dma_start(
                    out=g[32 * j : 32 * (j + 1), :],
                    out_offset=None,
                    in_=embw[:],
                    in_offset=bass.IndirectOffsetOnAxis(ap=idx[:, j : j + 1], axis=0),
                )

            # ---- node embedding as columns z01[128, 2] (PE transposes) ----
            z01 = sb.tile([128, 2], F32)
            for j in range(2):
                p = ps.tile([128, 1], F32, tag=f"t{j}")
                nc.tensor.transpose(
                    out=p[:], in_=cc[0:1, 128 * j : 128 * (j + 1)],
                    identity=ident[0:1, 0:1],
                )
                nc.vector.tensor_copy(out=z01[:, j : j + 1], in_=p[:])

            # ---- A = node_cols.T @ Ww_top   (overlaps the gather) ----
            a_p = ps.tile([1, O], F32)
            nc.tensor.matmul(out=a_p[:], lhsT=z01[:, 0:1], rhs=ww_sb[0][:], start=True, stop=False)
            nc.tensor.matmul(out=a_p[:], lhsT=z01[:, 1:2], rhs=ww_sb[1][:], start=False, stop=True)

            # ---- w row: wsum -> rec = 1/(sum w + eps) ----
            wr_p = ps.tile([1, K], F32, tag="t0")
            nc.tensor.transpose(out=wr_p[:], in_=g[:, C : C + 1], identity=ident[:])
            wsum = sb.tile([1, 1], F32)
            nc.vector.reduce_sum(out=wsum[:], in_=wr_p[:], axis=mybir.AxisListType.X)
            den = sb.tile([1, 1], F32)
            nc.vector.tensor_scalar_add(den[:], wsum[:], EPS)
            rec = sb.tile([1, 1], F32)
            nc.vector.reciprocal(rec[:], den[:])

            # ---- gT chunks ----
            gt = []
            for j in range(2):
                p = ps.tile([128, K], F32, tag=f"t{j}")
                nc.tensor.transpose(
                    out=p[:], in_=g[:, 128 * j : 128 * (j + 1)], identity=ident[:]
                )
                s = sb.tile([128, K], F32, tag=f"gts{j}")
                nc.vector.tensor_copy(out=s[:], in_=p[:])
                gt.append(s)

            # ---- h_pre = gT.T @ Qw (+ Qb) ----
            h_p = ps.tile([K, H], F32)
            nc.tensor.matmul(out=h_p[:], lhsT=gt[0][:], rhs=qw_sb[0][:], start=True, stop=False)
            nc.tensor.matmul(
                out=h_p[:], lhsT=gt[1][:], rhs=qw_sb[1][:],
                start=False, stop=not has_qb,
            )
            if has_qb:
                nc.tensor.matmul(out=h_p[:], lhsT=ones_r[:], rhs=qb_r[:], start=False, stop=True)

            # ---- leaky relu (DVE) ----
            ht = sb.tile([K, H], F32)
            nc.vector.tensor_scalar_mul(ht[:], h_p[:], ALPHA)
            h_l = sb.tile([K, H], F32)
            nc.vector.tensor_tensor(out=h_l[:], in0=h_p[:], in1=ht[:], op=MAX)

            # ---- s_raw cols = h.T @ w_col (unnormalized) ----
            z23 = sb.tile([128, 2], F32)
            for j in range(2):
                p = ps.tile([128, 1], F32, tag=f"t{j}")
                nc.tensor.matmul(
                    out=p[:], lhsT=h_l[:, 128 * j : 128 * (j + 1)],
                    rhs=g[:, C : C + 1], start=True, stop=True,
                )
                nc.vector.tensor_copy(out=z23[:, j : j + 1], in_=p[:])

            # ---- B = s_cols.T @ Ww_bot ----
            b_p = ps.tile([1, O], F32)
            nc.tensor.matmul(out=b_p[:], lhsT=z23[:, 0:1], rhs=ww_sb[2][:], start=True, stop=False)
            nc.tensor.matmul(out=b_p[:], lhsT=z23[:, 1:2], rhs=ww_sb[3][:], start=False, stop=True)

            # ---- combine: x = A + Wb + rec*B ; o = leaky(x) ----
            t1 = sb.tile([1, O], F32)
            nc.vector.tensor_scalar_mul(t1[:], b_p[:], rec[:])
            t2 = sb.tile([1, O], F32)
            nc.vector.tensor_tensor(out=t2[:], in0=a_p[:], in1=t1[:], op=ADD)
            x = sb.tile([1, O], F32)
            nc.vector.tensor_tensor(out=x[:], in0=t2[:], in1=wb_r[:], op=ADD)
            xt = sb.tile([1, O], F32)
            nc.vector.tensor_scalar_mul(xt[:], x[:], ALPHA)
            o2 = sb.tile([1, O], F32)
            nc.vector.tensor_tensor(out=o2[:], in0=x[:], in1=xt[:], op=MAX)

            # ---- L2 normalize ----
            sq = sb.tile([1, O], F32)
            nc.vector.tensor_tensor(out=sq[:], in0=o2[:], in1=o2[:], op=MULT)
            n2 = sb.tile([1, 1], F32)
            nc.vector.reduce_sum(out=n2[:], in_=sq[:], axis=mybir.AxisListType.X)
            nrm = sb.tile([1, 1], F32)
            nc.scalar.activation(
                out=nrm[:], in_=n2[:], func=mybir.ActivationFunctionType.Sqrt
            )
            den2 = sb.tile([1, 1], F32)
            nc.vector.tensor_scalar_add(den2[:], nrm[:], EPS)
            rec2 = sb.tile([1, 1], F32)
            nc.vector.reciprocal(rec2[:], den2[:])
            res = sb.tile([1, O], F32)
            nc.vector.tensor_scalar_mul(res[:], o2[:], rec2[:])

            nc.sync.dma_start(out=out_d[:], in_=res[:])

    nc.finalize()
    return nc


@functools.lru_cache(maxsize=4)
def _program(node_id: int, has_qb: bool) -> bass.Bass:
    return _build_program(node_id, has_qb)


def kernel(
    embeddings: np.ndarray,
    weights: np.ndarray,
    Qw: np.ndarray,
    Qb: np.ndarray,
    Ww: np.ndarray,
    Wb: np.ndarray,
    neighbor_set: np.ndarray,
    node_id,
    _trace: bool = False,
):
    node_id = int(np.asarray(node_id))
    nbr = np.asarray(neighbor_set).astype(np.int32)
    nbr_packed = np.ascontiguousarray(nbr.reshape(2, 32).T)
    wcol = np.asarray(weights[:, node_id], dtype=np.float32).reshape(N, 1)
    qw = np.ascontiguousarray(Qw, dtype=np.float32)
    qb = np.ascontiguousarray(Qb, dtype=np.float32).reshape(1, H)
    ww = np.ascontiguousarray(Ww, dtype=np.float32)
    wb = np.ascontiguousarray(Wb, dtype=np.float32).reshape(1, O)
    ident = np.eye(K, dtype=np.float32)
    has_qb = bool(np.any(qb))

    nc = _program(node_id, has_qb)
    in_maps = []
    for b in range(N_CORES):
        m = {
            "embw": np.concatenate(
                [np.asarray(embeddings[b], dtype=np.float32), wcol], axis=1
            ),
            "qw": qw,
            "ww": ww,
            "wb": wb,
            "nbr": nbr_packed,
            "ident": ident,
        }
        if has_qb:
            m["qb"] = qb
            m["onesr"] = np.ones((1, K), dtype=np.float32)
        in_maps.append(m)
    r = run_bass_kernel_spmd(nc, in_maps, list(range(N_CORES)), trace=_trace)
    out = np.stack([r.results[b]["out"][0] for b in range(N_CORES)], axis=0)
    if _trace:
        return out, r
    return out


# revision 11
# speedup vs baseline: 1.1101x; 1.1101x over previous
"""GNN message-passing (Convolve) kernel for Trainium2, 8 NeuronCores.

Reference computation (B=8, N=8192, C=256, H=256, O=256, K=64):
    g   = embeddings[:, neighbor_set, :]                     # [B, K, C]
    h   = leaky_relu(g @ Qw + Qb)                            # [B, K, H]
    w   = weights[neighbor_set, node_id]                     # [K]
    s   = sum_k h * w / (sum_k w + eps)                      # [B, H]
    z   = concat(embeddings[:, node_id, :], s)               # [B, C+H]
    o   = leaky_relu(z @ Ww + Wb)                            # [B, O]
    out = o / (||o||_2 + eps)                                # [B, O]

Sharding: data-parallel over the batch axis — core b handles batch b.
Each core receives an augmented table T = [embeddings[b] | weights[:, node_id]]
([N, C+1]) so one indirect-DMA gather fetches both the neighbor embedding
row and its edge weight.  Qw/Ww/biases are replicated.

Device dataflow (fp32; leaky-relu/square/sqrt on the pre-warmed ACT engine):
    [window] node cols via PE transposes; A[1,256] = node.T @ Ww_top
    gather g[64, 257] (one indirect DMA, 16-queue fanout)
    gT chunks via PE transposes; wsum via ones.T @ w_col matmul
    h_pre = gT.T @ Qw (+ ones.T @ Qb if Qb nonzero); h = Prelu(h_pre)
    s_raw cols = h[:, chunk].T @ w_col; B = s_cols.T @ Ww_bot
    x = (A + Wb) + B/(wsum+eps); o = Prelu(x)
    out = o / (sqrt(sum o^2) + eps)
"""

import functools

import numpy as np

import concourse.bacc as bacc
import concourse.bass as bass
import concourse.mybir as mybir
import concourse.tile as tile
from concourse.bass_utils import run_bass_kernel_spmd

B, N, C, H, O, K = 8, 8192, 256, 256, 256, 64
ALPHA = 0.3
EPS = 1e-6
F32 = mybir.dt.float32
I32 = mybir.dt.int32
N_CORES = 8
MAX = mybir.AluOpType.max
MULT = mybir.AluOpType.mult
ADD = mybir.AluOpType.add
AF = mybir.ActivationFunctionType


def _build_program(node_id: int, has_qb: bool) -> bass.Bass:
    nc = bacc.Bacc(None, target_bir_lowering=False, debug=False)

    embw = nc.dram_tensor("embw", [N, C + 1], F32, kind="ExternalInput")
    qw = nc.dram_tensor("qw", [C, H], F32, kind="ExternalInput")
    ww = nc.dram_tensor("ww", [C + H, O], F32, kind="ExternalInput")
    wb = nc.dram_tensor("wb", [1, O], F32, kind="ExternalInput")
    nbr = nc.dram_tensor("nbr", [K, 1], I32, kind="ExternalInput")
    ident_d = nc.dram_tensor("ident", [K, K + 1], F32, kind="ExternalInput")
    if has_qb:
        qb = nc.dram_tensor("qb", [1, H], F32, kind="ExternalInput")
        onesr_d = nc.dram_tensor("onesr", [1, K], F32, kind="ExternalInput")
    out_d = nc.dram_tensor("out", [1, O], F32, kind="ExternalOutput")

    with tile.TileContext(nc) as tc:
        with (
            tc.tile_pool(name="sb", bufs=1) as sb,
            tc.tile_pool(name="ps", bufs=1, space="PSUM") as ps,
        ):
            # ---- DMA issue spread over the two HWDGE sequencers ----
            # sync: Ww halves (A-group dep first), idx, Wb
            ww01 = sb.tile([128, 512], F32)
            nc.sync.dma_start(
                out=ww01[:].rearrange("p (two o) -> p two o", two=2),
                in_=ww[0:256, :].rearrange("(two p) o -> p two o", two=2),
            )
            idx = sb.tile([K, 1], I32)
            nc.sync.dma_start(out=idx[:], in_=nbr[:])
            ww23 = sb.tile([128, 512], F32)
            nc.sync.dma_start(
                out=ww23[:].rearrange("p (two o) -> p two o", two=2),
                in_=ww[256:512, :].rearrange("(two p) o -> p two o", two=2),
            )
            wb_r = sb.tile([1, O], F32)
            nc.sync.dma_start(out=wb_r[:], in_=wb[:])
            # scalar/ACT: identity+ones, node row, Qw fused, then table warm
            ident = sb.tile([K, K + 1], F32)
            nc.scalar.dma_start(out=ident[:], in_=ident_d[:])
            cc = sb.tile([1, C], F32)
            nc.scalar.dma_start(out=cc[:], in_=embw[node_id : node_id + 1, 0:C])
            qw_f = sb.tile([128, 512], F32)
            nc.scalar.dma_start(
                out=qw_f[:].rearrange("p (two h) -> p two h", two=2),
                in_=qw[:].rearrange("(two p) h -> p two h", two=2),
            )
            if has_qb:
                qb_r = sb.tile([1, H], F32)
                nc.scalar.dma_start(out=qb_r[:], in_=qb[:])
                ones_r = sb.tile([1, K], F32)
                nc.scalar.dma_start(out=ones_r[:], in_=onesr_d[:])
            warm = sb.tile([1, 1], F32)
            nc.scalar.activation(out=warm[:], in_=ident[0:1, 0:1], func=AF.Sqrt)

            # ---- gather: g[k, :] = embw[idx[k], :] ----
            g = sb.tile([K, C + 1], F32)
            nc.gpsimd.indirect_dma_start(
                out=g[:],
                out_offset=None,
                in_=embw[:],
                in_offset=bass.IndirectOffsetOnAxis(ap=idx[:, :1], axis=0),
            )

            # ---- window work: node cols + A = node.T @ Ww_top ----
            z01 = sb.tile([128, 2], F32)
            for j in range(2):
                p = ps.tile([128, 1], F32, tag=f"t{j}")
                nc.tensor.transpose(
                    out=p[:], in_=cc[0:1, 128 * j : 128 * (j + 1)],
                    identity=ident[0:1, 0:1],
                )
                nc.vector.tensor_copy(out=z01[:, j : j + 1], in_=p[:])
            a_p = ps.tile([1, O], F32)
            nc.tensor.matmul(out=a_p[:], lhsT=z01[:, 0:1], rhs=ww01[:, 0:256], start=True, stop=False)
            nc.tensor.matmul(out=a_p[:], lhsT=z01[:, 1:2], rhs=ww01[:, 256:512], start=False, stop=True)

            # ---- gT chunks + wsum ----
            gt = []
            for j in range(2):
                p = ps.tile([128, K], F32, tag=f"t{j}")
                nc.tensor.transpose(
                    out=p[:], in_=g[:, 128 * j : 128 * (j + 1)],
                    identity=ident[:, 0:K],
                )
                s = sb.tile([128, K], F32, tag=f"gts{j}")
                nc.vector.tensor_copy(out=s[:], in_=p[:])
                gt.append(s)
            se_p = ps.tile([1, 1], F32)
            nc.tensor.matmul(
                out=se_p[:], lhsT=ident[:, K : K + 1], rhs=g[:, C : C + 1],
                start=True, stop=True,
            )
            den = sb.tile([1, 1], F32)
            nc.vector.tensor_scalar_add(den[:], se_p[:], EPS)
            rec = sb.tile([1, 1], F32)
            nc.vector.reciprocal(rec[:], den[:])

            # ---- h = Prelu(gT.T @ Qw (+ Qb)) ----
            h_p = ps.tile([K, H], F32)
            nc.tensor.matmul(out=h_p[:], lhsT=gt[0][:], rhs=qw_f[:, 0:256], start=True, stop=False)
            nc.tensor.matmul(
                out=h_p[:], lhsT=gt[1][:], rhs=qw_f[:, 256:512],
                start=False, stop=not has_qb,
            )
            if has_qb:
                nc.tensor.matmul(
                    out=h_p[:], lhsT=ones_r[:], rhs=qb_r[:], start=False, stop=True,
                )
            h_l = sb.tile([K, H], F32)
            nc.scalar.activation(out=h_l[:], in_=h_p[:], func=AF.Prelu, alpha=ALPHA)

            # ---- s_raw cols = h.T @ w_col ----
            z23 = sb.tile([128, 2], F32)
            for j in range(2):
                p = ps.tile([128, 1], F32, tag=f"t{j}")
                nc.tensor.matmul(
                    out=p[:], lhsT=h_l[:, 128 * j : 128 * (j + 1)],
                    rhs=g[:, C : C + 1], start=True, stop=True,
                )
                nc.vector.tensor_copy(out=z23[:, j : j + 1], in_=p[:])

            # ---- B = s_cols.T @ Ww_bot ----
            b_p = ps.tile([1, O], F32)
            nc.tensor.matmul(out=b_p[:], lhsT=z23[:, 0:1], rhs=ww23[:, 0:256], start=True, stop=False)
            nc.tensor.matmul(out=b_p[:], lhsT=z23[:, 1:2], rhs=ww23[:, 256:512], start=False, stop=True)

            # ---- combine + leaky + L2 normalize ----
            t_a = sb.tile([1, O], F32)
            nc.vector.tensor_tensor(out=t_a[:], in0=a_p[:], in1=wb_r[:], op=ADD)
            t1 = sb.tile([1, O], F32)
            nc.vector.tensor_scalar_mul(t1[:], b_p[:], rec[:])
            x = sb.tile([1, O], F32)
            nc.vector.tensor_tensor(out=x[:], in0=t_a[:], in1=t1[:], op=ADD)
            o2 = sb.tile([1, O], F32)
            nc.scalar.activation(out=o2[:], in_=x[:], func=AF.Prelu, alpha=ALPHA)
            sq = sb.tile([1, O], F32)
            n2 = sb.tile([1, 1], F32)
            nc.scalar.activation(out=sq[:], in_=o2[:], func=AF.Square, accum_out=n2[:])
            nrm = sb.tile([1, 1], F32)
            nc.scalar.activation(out=nrm[:], in_=n2[:], func=AF.Sqrt)
            den2 = sb.tile([1, 1], F32)
            nc.vector.tensor_scalar_add(den2[:], nrm[:], EPS)
            rec2 = sb.tile([1, 1], F32)
            nc.vector.reciprocal(rec2[:], den2[:])
            res = sb.tile([1, O], F32)
            nc.vector.tensor_scalar_mul(res[:], o2[:], rec2[:])

            nc.sync.dma_start(out=out_d[:], in_=res[:])

    nc.finalize()
    return nc


@functools.lru_cache(maxsize=4)
def _program(node_id: int, has_qb: bool) -> bass.Bass:
    return _build_program(node_id, has_qb)


def kernel(
    embeddings: np.ndarray,
    weights: np.ndarray,
    Qw: np.ndarray,
    Qb: np.ndarray,
    Ww: np.ndarray,
    Wb: np.ndarray,
    neighbor_set: np.ndarray,
    node_id,
    _trace: bool = False,
):
    node_id = int(np.asarray(node_id))
    nbr = np.ascontiguousarray(
        np.asarray(neighbor_set).astype(np.int32).reshape(K, 1)
    )
    wcol = np.asarray(weights[:, node_id], dtype=np.float32).reshape(N, 1)
    qw = np.ascontiguousarray(Qw, dtype=np.float32)
    qb = np.ascontiguousarray(Qb, dtype=np.float32).reshape(1, H)
    ww = np.ascontiguousarray(Ww, dtype=np.float32)
    wb = np.ascontiguousarray(Wb, dtype=np.float32).reshape(1, O)
    ident = np.concatenate(
        [np.eye(K, dtype=np.float32), np.ones((K, 1), dtype=np.float32)], axis=1
    )
    has_qb = bool(np.any(qb))

    nc = _program(node_id, has_qb)
    in_maps = []
    for b in range(N_CORES):
        m = {
            "embw": np.concatenate(
                [np.asarray(embeddings[b], dtype=np.float32), wcol], axis=1
            ),
            "qw": qw,
            "ww": ww,
            "wb": wb,
            "nbr": nbr,
            "ident": ident,
        }
        if has_qb:
            m["qb"] = qb
            m["onesr"] = np.ones((1, K), dtype=np.float32)
        in_maps.append(m)
    r = run_bass_kernel_spmd(nc, in_maps, list(range(N_CORES)), trace=_trace)
    out = np.stack([r.results[b]["out"][0] for b in range(N_CORES)], axis=0)
    if _trace:
        return out, r
    return out


# revision 12
# speedup vs baseline: 1.1181x; 1.0072x over previous
"""GNN message-passing (Convolve) kernel for Trainium2, 8 NeuronCores.

Reference computation (B=8, N=8192, C=256, H=256, O=256, K=64):
    g   = embeddings[:, neighbor_set, :]                     # [B, K, C]
    h   = leaky_relu(g @ Qw + Qb)                            # [B, K, H]
    w   = weights[neighbor_set, node_id]                     # [K]
    s   = sum_k h * w / (sum_k w + eps)                      # [B, H]
    z   = concat(embeddings[:, node_id, :], s)               # [B, C+H]
    o   = leaky_relu(z @ Ww + Wb)                            # [B, O]
    out = o / (||o||_2 + eps)                                # [B, O]

Sharding: data-parallel over the batch axis — core b handles batch b.
Each core receives an augmented table T = [embeddings[b] | weights[:, node_id]]
([N, C+1]) so one indirect-DMA gather fetches both the neighbor embedding
row and its edge weight.  Qw/Ww/biases are replicated.

Device dataflow (fp32):
    constants (identity/ones) built on gpsimd while its DMA library warms
    gather g[64, 257] (one indirect DMA, 16-queue fanout)
    [gather window] node cols via PE transposes; x_p = node.T @ Ww_top
    den_col[64,1] = ones_mat.T @ w_col (+eps, 1/x on DVE) -> wn = w * rec
    h = Prelu(gT.T @ Qw (+ Qb)); s cols = h.T @ wn  (normalized)
    x_p += s_cols.T @ Ww_bot   (same PSUM accumulation group as node part)
    o = Prelu(x_p + Wb); out = o / (sqrt(sum o^2) + eps)   (warm ACT)
"""

import functools

import numpy as np

import concourse.bacc as bacc
import concourse.bass as bass
import concourse.mybir as mybir
import concourse.tile as tile
from concourse.bass_utils import run_bass_kernel_spmd
from concourse.masks import make_identity

B, N, C, H, O, K = 8, 8192, 256, 256, 256, 64
ALPHA = 0.3
EPS = 1e-6
F32 = mybir.dt.float32
I32 = mybir.dt.int32
N_CORES = 8
MULT = mybir.AluOpType.mult
ADD = mybir.AluOpType.add
AF = mybir.ActivationFunctionType


def _build_program(node_id: int, has_qb: bool) -> bass.Bass:
    nc = bacc.Bacc(None, target_bir_lowering=False, debug=False)

    embw = nc.dram_tensor("embw", [N, C + 1], F32, kind="ExternalInput")
    qw = nc.dram_tensor("qw", [C, H], F32, kind="ExternalInput")
    ww = nc.dram_tensor("ww", [C + H, O], F32, kind="ExternalInput")
    wb = nc.dram_tensor("wb", [1, O], F32, kind="ExternalInput")
    nbr = nc.dram_tensor("nbr", [K, 1], I32, kind="ExternalInput")
    if has_qb:
        qb = nc.dram_tensor("qb", [1, H], F32, kind="ExternalInput")
    out_d = nc.dram_tensor("out", [1, O], F32, kind="ExternalOutput")

    with tile.TileContext(nc) as tc:
        with (
            tc.tile_pool(name="sb", bufs=1) as sb,
            tc.tile_pool(name="ps", bufs=1, space="PSUM") as ps,
        ):
            # ---- sync HWDGE: idx first (gates gather), then weights ----
            idx = sb.tile([K, 1], I32)
            nc.sync.dma_start(out=idx[:], in_=nbr[:])
            ww01 = sb.tile([128, 512], F32)
            nc.sync.dma_start(
                out=ww01[:].rearrange("p (two o) -> p two o", two=2),
                in_=ww[0:256, :].rearrange("(two p) o -> p two o", two=2),
            )
            # fused [Qw ; Ww_bot] -> [128, 1024]
            w2 = sb.tile([128, 1024], F32)
            nc.sync.dma_start(
                out=w2[:, 0:512].rearrange("p (two h) -> p two h", two=2),
                in_=qw[:].rearrange("(two p) h -> p two h", two=2),
            )
            nc.sync.dma_start(
                out=w2[:, 512:1024].rearrange("p (two o) -> p two o", two=2),
                in_=ww[256:512, :].rearrange("(two p) o -> p two o", two=2),
            )
            wb_r = sb.tile([1, O], F32)
            nc.sync.dma_start(out=wb_r[:], in_=wb[:])
            # ---- scalar/ACT HWDGE: node row, bias, table warm ----
            cc = sb.tile([1, C], F32)
            nc.scalar.dma_start(out=cc[:], in_=embw[node_id : node_id + 1, 0:C])
            if has_qb:
                qb_r = sb.tile([1, H], F32)
                nc.scalar.dma_start(out=qb_r[:], in_=qb[:])
            warm1 = sb.tile([1, 1], F32)
            nc.scalar.activation(out=warm1[:], in_=cc[0:1, 0:1], func=AF.Square)
            warm2 = sb.tile([1, 1], F32)
            nc.scalar.activation(out=warm2[:], in_=warm1[:], func=AF.Sqrt)

            # ---- gpsimd: constants (before its DMA-library branch) ----
            # cb = [ eye(64) | ones[64,1] | ones[64,64] ]
            cb = sb.tile([K, 2 * K + 1], F32)
            make_identity(nc, cb[:, 0:K])
            nc.gpsimd.memset(cb[:, K : 2 * K + 1], 1.0)

            # ---- gather: g[k, :] = embw[idx[k], :] ----
            g = sb.tile([K, C + 1], F32)
            nc.gpsimd.indirect_dma_start(
                out=g[:],
                out_offset=None,
                in_=embw[:],
                in_offset=bass.IndirectOffsetOnAxis(ap=idx[:, :1], axis=0),
            )

            # ---- window: node cols; x_p = node.T @ Ww_top (group opens) ----
            z01 = sb.tile([128, 2], F32)
            for j in range(2):
                p = ps.tile([128, 1], F32, tag=f"t{j}")
                nc.tensor.transpose(
                    out=p[:], in_=cc[0:1, 128 * j : 128 * (j + 1)],
                    identity=cb[0:1, 0:1],
                )
                nc.vector.tensor_copy(out=z01[:, j : j + 1], in_=p[:])
            x_p = ps.tile([1, O], F32)
            nc.tensor.matmul(
                out=x_p[:], lhsT=z01[:, 0:1], rhs=ww01[:, 0:256],
                start=True, stop=False, skip_group_check=True,
            )
            nc.tensor.matmul(
                out=x_p[:], lhsT=z01[:, 1:2], rhs=ww01[:, 256:512],
                start=False, stop=False, skip_group_check=True,
            )

            # ---- gT chunks; den_col = ones_mat.T @ w_col ----
            gt = []
            for j in range(2):
                p = ps.tile([128, K], F32, tag=f"t{j}")
                nc.tensor.transpose(
                    out=p[:], in_=g[:, 128 * j : 128 * (j + 1)],
                    identity=cb[:, 0:K],
                )
                s = sb.tile([128, K], F32, tag=f"gts{j}")
                nc.vector.tensor_copy(out=s[:], in_=p[:])
                gt.append(s)
            dc_p = ps.tile([K, 1], F32, tag="t0")
            nc.tensor.matmul(
                out=dc_p[:], lhsT=cb[:, K + 1 : 2 * K + 1], rhs=g[:, C : C + 1],
                start=True, stop=True,
            )
            dc = sb.tile([K, 1], F32)
            nc.vector.tensor_scalar_add(dc[:], dc_p[:], EPS)
            rc = sb.tile([K, 1], F32)
            nc.vector.reciprocal(rc[:], dc[:])
            wn = sb.tile([K, 1], F32)
            nc.vector.tensor_tensor(out=wn[:], in0=g[:, C : C + 1], in1=rc[:], op=MULT)

            # ---- h = Prelu(gT.T @ Qw (+ Qb)) ----
            h_p = ps.tile([K, H], F32)
            nc.tensor.matmul(out=h_p[:], lhsT=gt[0][:], rhs=w2[:, 0:256], start=True, stop=False)
            nc.tensor.matmul(
                out=h_p[:], lhsT=gt[1][:], rhs=w2[:, 256:512],
                start=False, stop=not has_qb,
            )
            if has_qb:
                ones_p = ps.tile([1, K], F32, tag="t1")
                nc.tensor.transpose(out=ones_p[:], in_=cb[:, K : K + 1], identity=cb[:, 0:K])
                ones_r = sb.tile([1, K], F32)
                nc.vector.tensor_copy(out=ones_r[:], in_=ones_p[:])
                nc.tensor.matmul(
                    out=h_p[:], lhsT=ones_r[:], rhs=qb_r[:], start=False, stop=True,
                )
            h_l = sb.tile([K, H], F32)
            nc.scalar.activation(out=h_l[:], in_=h_p[:], func=AF.Prelu, alpha=ALPHA)

            # ---- s cols (normalized) = h.T @ wn; x_p += s.T @ Ww_bot ----
            z23 = sb.tile([128, 2], F32)
            for j in range(2):
                p = ps.tile([128, 1], F32, tag=f"t{j}")
                nc.tensor.matmul(
                    out=p[:], lhsT=h_l[:, 128 * j : 128 * (j + 1)],
                    rhs=wn[:], start=True, stop=True,
                )
                nc.vector.tensor_copy(out=z23[:, j : j + 1], in_=p[:])
            nc.tensor.matmul(
                out=x_p[:], lhsT=z23[:, 0:1], rhs=w2[:, 512:768],
                start=False, stop=False, skip_group_check=True,
            )
            nc.tensor.matmul(
                out=x_p[:], lhsT=z23[:, 1:2], rhs=w2[:, 768:1024],
                start=False, stop=True, skip_group_check=True,
            )

            # ---- o = Prelu(x_p + Wb); out = o/(sqrt(sum o^2)+eps) ----
            x = sb.tile([1, O], F32)
            nc.vector.tensor_tensor(out=x[:], in0=x_p[:], in1=wb_r[:], op=ADD)
            o2 = sb.tile([1, O], F32)
            nc.scalar.activation(out=o2[:], in_=x[:], func=AF.Prelu, alpha=ALPHA)
            sq = sb.tile([1, O], F32)
            n2 = sb.tile([1, 1], F32)
            nc.scalar.activation(out=sq[:], in_=o2[:], func=AF.Square, accum_out=n2[:])
            nrm = sb.tile([1, 1], F32)
            nc.scalar.activation(out=nrm[:], in_=n2[:], func=AF.Sqrt)
            den2 = sb.tile([1, 1], F32)
            nc.vector.tensor_scalar_add(den2[:], nrm[:], EPS)
            rec2 = sb.tile([1, 1], F32)
            nc.vector.reciprocal(rec2[:], den2[:])
            res = sb.tile([1, O], F32)
            nc.vector.tensor_scalar_mul(res[:], o2[:], rec2[:])

            nc.sync.dma_start(out=out_d[:], in_=res[:])

    nc.finalize()
    return nc


@functools.lru_cache(maxsize=4)
def _program(node_id: int, has_qb: bool) -> bass.Bass:
    return _build_program(node_id, has_qb)


def kernel(
    embeddings: np.ndarray,
    weights: np.ndarray,
    Qw: np.ndarray,
    Qb: np.ndarray,
    Ww: np.ndarray,
    Wb: np.ndarray,
    neighbor_set: np.ndarray,
    node_id,
    _trace: bool = False,
):
    node_id = int(np.asarray(node_id))
    nbr = np.ascontiguousarray(
        np.asarray(neighbor_set).astype(np.int32).reshape(K, 1)
    )
    wcol = np.asarray(weights[:, node_id], dtype=np.float32).reshape(N, 1)
    qw = np.ascontiguousarray(Qw, dtype=np.float32)
    qb = np.ascontiguousarray(Qb, dtype=np.float32).reshape(1, H)
    ww = np.ascontiguousarray(Ww, dtype=np.float32)
    wb = np.ascontiguousarray(Wb, dtype=np.float32).reshape(1, O)
    has_qb = bool(np.any(qb))

    nc = _program(node_id, has_qb)
    in_maps = []
    for b in range(N_CORES):
        m = {
            "embw": np.concatenate(
                [np.asarray(embeddings[b], dtype=np.float32), wcol], axis=1
            ),
            "qw": qw,
            "ww": ww,
            "wb": wb,
            "nbr": nbr,
        }
        if has_qb:
            m["qb"] = qb
        in_maps.append(m)
    r = run_bass_kernel_spmd(nc, in_maps, list(range(N_CORES)), trace=_trace)
    out = np.stack([r.results[b]["out"][0] for b in range(N_CORES)], axis=0)
    if _trace:
        return out, r
    return out
